# revision 24
# baseline (speedup 1.0000x reference)
"""Trainium2 Bass kernel for multi-head attention (B=4, N=2048, C=256, H=16).

Sharding: 8 cores, each core handles one batch b = core//2 and 8 heads
(half of 16) g = core%2.  Each core computes its 8 heads' attention plus a
partial output projection (its heads' rows of w_proj); the host sums the
two partials per batch and adds b_proj.

Per-core on-chip algorithm (all layouts "transposed", channels on
partitions):
  xT   = x_b^T                        via PE transpose        [C, N]
  qT/kT (spread layout: head j of a 4-head group occupies partitions
        32j..32j+16) = W^T @ xT                               [128, N]
  vT   (compact: head lh at partitions 16lh)                  [128, N]
  v_aug[keys, lh, 0:16] = v, v_aug[keys, lh, 16] = 1          (ones col
        makes the attn@v matmul also produce softmax row-sums)
  S^T  = k_h @ q_h^T   (row-group-packed matmuls, K=16)       [keys, q]
  P^T  = exp(S^T)      (ScalarE, PSUM->SBUF; no max subtraction needed:
        |logits| <= ~45 so exp stays in fp32 range)
  outT_aug = v_aug^T @ P^T  accumulated over key tiles in PSUM; row 16 of
        each 32-row col-group = sum_j P^T[j, q]  (softmax denominator)
  bc   = Sel^T @ outT  broadcasts each group's sum row over the group
  outT_norm = outT * reciprocal(bc)
  partial = outT_norm^T @ Wp_spread   (zero rows kill sum/garbage rows)

Matmul dtypes: fp32r (TF32-like, 4x the fp32 PE rate) for qkv/scores/
attnv; plain fp32 for the sum-broadcast and output projection.  Walrus
requires every producer of an fp32r matmul operand to emit fp32r, so the
operand tiles (and the weight DRAM tensors) are declared float32r.
"""

import numpy as np

import concourse.bass as bass
import concourse.mybir as mybir
import concourse.tile as tile
from concourse import bacc

F32 = mybir.dt.float32
F32R = mybir.dt.float32r
BF16 = mybir.dt.bfloat16
EXPF = mybir.ActivationFunctionType.Exp

P = 128
B, N_FULL, C, H, D = 4, 2048, 256, 16, 16
CC = C // P  # 2 channel tiles
NCORES = 8

# dtype knobs: "f32r" | "f32" | "bf16" per stage.
MM_DT = "f32r"    # qkv projection + scores matmuls
AV_DT = "bf16"    # attnv (P^T @ v_aug) matmuls: needs col-group tile_position,
                  # which the fp32r self-loading weight path cannot encode
PROJ_DT = "f32"   # sum-broadcast + output projection matmuls

_DT = {"f32r": F32R, "f32": F32, "bf16": BF16}

_NC_CACHE: dict = {}
LAST_RESULT = None  # BassKernelResults of the most recent run (for test.py)


def build(n_tokens=N_FULL, mm_dt=MM_DT, av_dt=AV_DT, proj_dt=PROJ_DT):
    N = n_tokens
    KT = N // P   # key tiles
    QC = 512      # q-chunk (psum bank = 512 fp32)
    NQ = N // QC
    TT = N // P   # token tiles

    MD = _DT[mm_dt]
    AD = _DT[av_dt]
    PD = _DT[proj_dt]

    # Bacc (not plain Bass): its compile() pass splits multi-semaphore
    # waits via EventSemaphore instructions — TPB instructions carry at
    # most one hardware wait slot.
    nc = bacc.Bacc()
    x_d = nc.dram_tensor("x", [N, C], F32, kind="ExternalInput")
    wq_d = nc.dram_tensor("wq", [2, C, P], MD, kind="ExternalInput")
    wk_d = nc.dram_tensor("wk", [2, C, P], MD, kind="ExternalInput")
    wv_d = nc.dram_tensor("wv", [C, P], MD, kind="ExternalInput")
    bq_d = nc.dram_tensor("bq", [2, P], F32, kind="ExternalInput")
    bk_d = nc.dram_tensor("bk", [2, P], F32, kind="ExternalInput")
    bv_d = nc.dram_tensor("bv", [P], F32, kind="ExternalInput")
    wp_d = nc.dram_tensor("wp", [2, P, C], PD, kind="ExternalInput")
    sel_d = nc.dram_tensor("sel", [P, P], PD, kind="ExternalInput")
    idn_d = nc.dram_tensor("idn", [P, P], F32, kind="ExternalInput")
    out_d = nc.dram_tensor("out", [N, C], F32, kind="ExternalOutput")

    with tile.TileContext(nc) as tc:
        with (
            tc.tile_pool(name="const", bufs=1) as const,
            tc.tile_pool(name="work", bufs=3) as work,
            tc.tile_pool(name="ptp", bufs=4) as ptp,
            tc.tile_pool(name="ps_s", bufs=2, space="PSUM") as ps_s,
            tc.tile_pool(name="ps_m", bufs=4, space="PSUM") as ps_m,
        ):
            # ---------------- loads ----------------
            # Tensors consumed by self-loading (fp32/fp32r/transpose)
            # matmuls are staged through one DVE copy: the fused LDWEIGHTS
            # carries the matmul's waits and has a tiny wait-command budget,
            # which direct multi-queue DMA producers overflow.
            def staged_load(name, shape, dt, src_ap):
                ld = const.tile(shape, dt, name=f"{name}_ld")
                nc.sync.dma_start(ld[:], src_ap)
                sb = const.tile(shape, dt, name=f"{name}_sb")
                nc.vector.tensor_copy(sb[:], ld[:])
                return sb

            x_sb = staged_load(
                "x", [P, TT, C], F32, x_d[:].rearrange("(t p) c -> p t c", p=P)
            )
            wq_sb = staged_load(
                "wq", [P, 2, CC, P], MD,
                wq_d[:].rearrange("g (cc p) f -> p g cc f", p=P),
            )
            wk_sb = staged_load(
                "wk", [P, 2, CC, P], MD,
                wk_d[:].rearrange("g (cc p) f -> p g cc f", p=P),
            )
            wv_sb = staged_load(
                "wv", [P, CC, P], MD, wv_d[:].rearrange("(cc p) f -> p cc f", p=P)
            )
            wp_sb = staged_load("wp", [P, 2, C], PD, wp_d[:].rearrange("g p c -> p g c"))
            sel_sb = staged_load("sel", [P, P], PD, sel_d[:])
            idn_sb = staged_load("idn", [P, P], F32, idn_d[:])
            bq_sb = staged_load("bq", [P, 2], F32, bq_d[:].rearrange("g p -> p g"))
            bk_sb = staged_load("bk", [P, 2], F32, bk_d[:].rearrange("g p -> p g"))
            bv_sb = staged_load(
                "bv", [P, 1], F32, bv_d[:].rearrange("(p o) -> p o", o=1)
            )

            # ---------------- xT via PE transpose ----------------
            xt_sb = const.tile([P, CC, N], MD)
            for tt in range(TT):
                for cc in range(CC):
                    tp = ps_m.tile([P, P], F32, tag="misc", name="tp")
                    nc.tensor.transpose(
                        tp[:], x_sb[:, tt, cc * P : (cc + 1) * P], idn_sb[:]
                    )
                    nc.vector.tensor_copy(xt_sb[:, cc, tt * P : (tt + 1) * P], tp[:])

            # ---------------- qkv projections ----------------
            qt_sb = const.tile([P, 2, N], MD)
            kt_sb = const.tile([P, 2, N], MD)
            vt_sb = const.tile([P, N], F32)
            for g2 in range(2):
                for w_sb, b_sb, dst in ((wq_sb, bq_sb, qt_sb), (wk_sb, bk_sb, kt_sb)):
                    for nn in range(N // QC):
                        ps = ps_m.tile([P, QC], F32, tag="misc", name="ps")
                        for cc in range(CC):
                            nc.tensor.matmul(
                                ps[:],
                                w_sb[:, g2, cc, :],
                                xt_sb[:, cc, nn * QC : (nn + 1) * QC],
                                start=(cc == 0),
                                stop=(cc == CC - 1),
                            )
                        # copy + in-place add: TensorScalar's ISA struct only
                        # fits one sync wait, so it must not read PSUM (PE
                        # wait) and carry its DVE pipeline wait at once
                        dslice = dst[:, g2, nn * QC : (nn + 1) * QC]
                        nc.vector.tensor_copy(dslice, ps[:])
                        nc.vector.tensor_scalar_add(dslice, dslice, b_sb[:, g2 : g2 + 1])
            for nn in range(N // QC):
                ps = ps_m.tile([P, QC], F32, tag="misc", name="ps")
                for cc in range(CC):
                    nc.tensor.matmul(
                        ps[:],
                        wv_sb[:, cc, :],
                        xt_sb[:, cc, nn * QC : (nn + 1) * QC],
                        start=(cc == 0),
                        stop=(cc == CC - 1),
                    )
                vslice = vt_sb[:, nn * QC : (nn + 1) * QC]
                nc.vector.tensor_copy(vslice, ps[:])
                nc.vector.tensor_scalar_add(vslice, vslice, bv_sb[:, 0:1])

            # ---------------- v_aug (v natural layout + ones column) -------
            vaug = const.tile([P, KT, 8, 17], AD)
            # fp32r memset has no ISA encoding; broadcast-copy 1.0 from an
            # fp32 tile instead (copies round to fp32r)
            ones_sb = const.tile([P, 1], F32)
            nc.vector.memset(ones_sb[:], 1.0)
            nc.vector.tensor_copy(
                vaug[:, :, :, 16], ones_sb[:, 0:1, None].to_broadcast((P, KT, 8))
            )
            for kt in range(KT):
                tp = ps_m.tile([P, P], F32, tag="misc", name="tp")
                nc.tensor.transpose(tp[:], vt_sb[:, kt * P : (kt + 1) * P], idn_sb[:])
                nc.vector.tensor_copy(
                    vaug[:, kt, :, 0:16], tp[:].rearrange("p (h d) -> p h d", d=16)
                )

            # ---------------- attention ----------------
            for nn in range(NQ):
                ot_n = work.tile([P, 2, QC], PD, tag="otn")
                for g2 in range(2):
                    # one accumulator bank per head: independent psum
                    # accumulation chains must not share a zero region
                    at = [
                        ps_m.tile([P, QC], F32, tag="misc", name=f"at{_lj}")
                        for _lj in range(4)
                    ]
                    for kt in range(KT):
                        for pr in range(2):
                            sc = ps_s.tile([P, 2 * QC], F32, tag="scores", name="sc")
                            for j2 in range(2):
                                lj = 2 * pr + j2
                                rg = 32 * lj
                                nc.tensor.matmul(
                                    sc[:, j2 * QC : (j2 + 1) * QC],
                                    kt_sb[rg : rg + D, g2, kt * P : (kt + 1) * P],
                                    qt_sb[rg : rg + D, g2, nn * QC : (nn + 1) * QC],
                                    start=True,
                                    stop=True,
                                    tile_position=(rg, 0),
                                )
                            pt = ptp.tile([P, 2 * QC], AD, tag="pt", name="pt")
                            nc.scalar.activation(pt[:], sc[:], EXPF)
                            for j2 in range(2):
                                lj = 2 * pr + j2
                                nc.tensor.matmul(
                                    at[lj][32 * lj : 32 * lj + 17, :],
                                    vaug[:, kt, 4 * g2 + lj, :],
                                    pt[:, j2 * QC : (j2 + 1) * QC],
                                    start=(kt == 0),
                                    stop=(kt == KT - 1),
                                    tile_position=(0, 32 * lj),
                                )
                    # normalize: broadcast sums over each col-group, divide
                    ot_raw = work.tile([P, QC], PD, tag="otraw")
                    nc.vector.memset(ot_raw[:], 0.0)
                    for lj in range(4):
                        nc.vector.tensor_copy(
                            ot_raw[32 * lj : 32 * lj + 17, :],
                            at[lj][32 * lj : 32 * lj + 17, :],
                        )
                    bc = ps_m.tile([P, QC], F32, tag="misc", name="bc")
                    nc.tensor.matmul(
                        bc[:], sel_sb[:], ot_raw[:], start=True, stop=True
                    )
                    rec = work.tile([P, QC], F32, tag="rec")
                    nc.vector.reciprocal(rec[:], bc[:])
                    nc.vector.tensor_mul(ot_n[:, g2, :], ot_raw[:], rec[:])
                # output projection for this q-chunk
                for ss in range(QC // P):
                    pp = ps_m.tile([P, C], F32, tag="misc", name="pp")
                    for g2 in range(2):
                        nc.tensor.matmul(
                            pp[:],
                            ot_n[:, g2, ss * P : (ss + 1) * P],
                            wp_sb[:, g2, :],
                            start=(g2 == 0),
                            stop=(g2 == 1),
                        )
                    ob = work.tile([P, C], F32, tag="ob")
                    nc.vector.tensor_copy(ob[:], pp[:])
                    tt_idx = nn * (QC // P) + ss
                    nc.sync.dma_start(
                        out_d[:].rearrange("(t p) c -> p t c", p=P)[:, tt_idx, :],
                        ob[:],
                    )
    nc.finalize()
    return nc


def _get_nc(n_tokens=N_FULL):
    key = (n_tokens, MM_DT, AV_DT, PROJ_DT)
    if key not in _NC_CACHE:
        _NC_CACHE[key] = build(n_tokens, MM_DT, AV_DT, PROJ_DT)
    return _NC_CACHE[key]


def make_core_inputs(core, x, w_qkv, b_qkv, w_proj, n_tokens=N_FULL):
    """Host-side sharding: slice/spread weights for one core."""
    b, g = core // 2, core % 2
    wq_s = np.zeros((2, C, P), np.float32)
    wk_s = np.zeros((2, C, P), np.float32)
    bq_s = np.zeros((2, P), np.float32)
    bk_s = np.zeros((2, P), np.float32)
    wv_s = np.zeros((C, P), np.float32)
    bv_s = np.zeros((P,), np.float32)
    wp_s = np.zeros((2, P, C), np.float32)
    for g2 in range(2):
        for j in range(4):
            h = 8 * g + 4 * g2 + j
            sp = slice(32 * j, 32 * j + D)
            wq_s[g2, :, sp] = w_qkv[:, 0 * C + h * D : 0 * C + (h + 1) * D]
            wk_s[g2, :, sp] = w_qkv[:, 1 * C + h * D : 1 * C + (h + 1) * D]
            bq_s[g2, sp] = b_qkv[0 * C + h * D : 0 * C + (h + 1) * D]
            bk_s[g2, sp] = b_qkv[1 * C + h * D : 1 * C + (h + 1) * D]
            wp_s[g2, sp, :] = w_proj[h * D : (h + 1) * D, :]
    for lh in range(8):
        h = 8 * g + lh
        wv_s[:, 16 * lh : 16 * lh + 16] = w_qkv[:, 2 * C + h * D : 2 * C + (h + 1) * D]
        bv_s[16 * lh : 16 * lh + 16] = b_qkv[2 * C + h * D : 2 * C + (h + 1) * D]
    sel = np.zeros((P, P), np.float32)
    for j in range(4):
        sel[32 * j + 16, 32 * j : 32 * j + 32] = 1.0
    idn = np.eye(P, dtype=np.float32)

    def cast(a, stage_dt):
        if stage_dt == "bf16":
            import ml_dtypes
            return a.astype(ml_dtypes.bfloat16)
        return a.astype(np.float32)

    return {
        "x": np.ascontiguousarray(x[b, :n_tokens], dtype=np.float32),
        "wq": cast(wq_s, MM_DT), "wk": cast(wk_s, MM_DT), "wv": cast(wv_s, MM_DT),
        "bq": bq_s, "bk": bk_s, "bv": bv_s,
        "wp": cast(wp_s, PROJ_DT), "sel": cast(sel, PROJ_DT), "idn": idn,
    }


def kernel(x, w_qkv, b_qkv, w_proj, b_proj):
    global LAST_RESULT
    from concourse.bass_utils import run_bass_kernel_spmd

    x = np.asarray(x, dtype=np.float32)
    w_qkv = np.asarray(w_qkv, dtype=np.float32)
    b_qkv = np.asarray(b_qkv, dtype=np.float32)
    w_proj = np.asarray(w_proj, dtype=np.float32)
    b_proj = np.asarray(b_proj, dtype=np.float32)

    nc = _get_nc()
    in_maps = [
        make_core_inputs(core, x, w_qkv, b_qkv, w_proj) for core in range(NCORES)
    ]
    res = run_bass_kernel_spmd(nc, in_maps, list(range(NCORES)))
    LAST_RESULT = res
    out = np.zeros((B, N_FULL, C), np.float32)
    for core in range(NCORES):
        out[core // 2] += res.results[core]["out"]
    out += b_proj[None, None, :]
    return out


# revision 35
# speedup vs baseline: 4083.5084x; 4083.5084x over previous
"""Trainium2 Bass kernel for multi-head attention (B=4, N=2048, C=256, H=16).

Sharding: 8 cores, each core handles one batch b = core//2 and 8 heads
(half of 16) g = core%2.  Each core computes its 8 heads' attention plus a
partial output projection (its heads' rows of w_proj); the host sums the
two partials per batch and adds b_proj.

Per-core on-chip algorithm (all layouts "transposed", channels on
partitions):
  xT   = x_b^T                        via PE transpose        [C, N]
  qT/kT (spread layout: head j of a 4-head group occupies partitions
        32j..32j+16) = W^T @ xT                               [128, N]
  vT   (compact: head lh at partitions 16lh)                  [128, N]
  v_aug[keys, lh, 0:16] = v, v_aug[keys, lh, 16] = 1          (ones col
        makes the attn@v matmul also produce softmax row-sums)
  S^T  = k_h @ q_h^T   (row-group-packed matmuls, K=16)       [keys, q]
  P^T  = exp(S^T)      (ScalarE, PSUM->SBUF; no max subtraction needed:
        |logits| <= ~45 so exp stays in fp32 range)
  outT_aug = v_aug^T @ P^T  accumulated over key tiles in PSUM; row 16 of
        each 32-row col-group = sum_j P^T[j, q]  (softmax denominator)
  bc   = Sel^T @ outT  broadcasts each group's sum row over the group
  outT_norm = outT * reciprocal(bc)
  partial = outT_norm^T @ Wp_spread   (zero rows kill sum/garbage rows)

Matmul dtypes: fp32r (TF32-like, 4x the fp32 PE rate) for qkv/scores and
the sum-broadcast/projection; bf16 for attnv (P^T is a probability
matrix, and the fp32r weight path cannot encode col-group tile_position).
Walrus requires every producer of an fp32r matmul operand to emit fp32r,
so the operand tiles (and the weight DRAM tensors) are declared float32r.
"""

import numpy as np

import concourse.bass as bass
import concourse.mybir as mybir
import concourse.tile as tile
from concourse import bacc

F32 = mybir.dt.float32
F32R = mybir.dt.float32r
BF16 = mybir.dt.bfloat16
EXPF = mybir.ActivationFunctionType.Exp

P = 128
B, N_FULL, C, H, D = 4, 2048, 256, 16, 16
CC = C // P  # 2 channel tiles
NCORES = 8

# dtype knobs: "f32r" | "f32" | "bf16" per stage.
MM_DT = "f32r"    # qkv projection + scores matmuls
AV_DT = "bf16"    # attnv (P^T @ v_aug) matmuls: needs col-group tile_position,
                  # which the fp32r self-loading weight path cannot encode
PROJ_DT = "f32r"  # sum-broadcast + output projection matmuls

_DT = {"f32r": F32R, "f32": F32, "bf16": BF16}

_NC_CACHE: dict = {}
LAST_RESULT = None  # BassKernelResults of the most recent run (for test.py)
TIMING_REPS = 1  # >1 repeats the compute on-device (timing); output unchanged


def build(n_tokens=N_FULL, mm_dt=MM_DT, av_dt=AV_DT, proj_dt=PROJ_DT, reps=1):
    N = n_tokens
    KT = N // P   # key tiles
    QC = 512      # q-chunk (psum bank = 512 fp32)
    NQ = N // QC
    TT = N // P   # token tiles

    MD = _DT[mm_dt]
    AD = _DT[av_dt]
    PD = _DT[proj_dt]

    # Bacc (not plain Bass): its compile() pass splits multi-semaphore
    # waits via EventSemaphore instructions — TPB instructions carry at
    # most one hardware wait slot.
    nc = bacc.Bacc()
    x_d = nc.dram_tensor("x", [N, C], F32, kind="ExternalInput")
    wq_d = nc.dram_tensor("wq", [2, C, P], MD, kind="ExternalInput")
    wk_d = nc.dram_tensor("wk", [2, C, P], MD, kind="ExternalInput")
    wv_d = nc.dram_tensor("wv", [C, P], MD, kind="ExternalInput")
    bq_d = nc.dram_tensor("bq", [2, P], F32, kind="ExternalInput")
    bk_d = nc.dram_tensor("bk", [2, P], F32, kind="ExternalInput")
    bv_d = nc.dram_tensor("bv", [P], F32, kind="ExternalInput")
    wp_d = nc.dram_tensor("wp", [2, P, C], PD, kind="ExternalInput")
    sel_d = nc.dram_tensor("sel", [P, P], PD, kind="ExternalInput")
    idn_d = nc.dram_tensor("idn", [P, P], F32, kind="ExternalInput")
    out_d = nc.dram_tensor("out", [N, C], F32, kind="ExternalOutput")

    with tile.TileContext(nc) as tc:
        with (
            tc.tile_pool(name="const", bufs=1) as const,
            tc.tile_pool(name="work", bufs=4) as work,
            tc.tile_pool(name="ptp", bufs=6) as ptp,
            tc.tile_pool(name="ps_s", bufs=2, space="PSUM") as ps_s,
            tc.tile_pool(name="ps_m", bufs=4, space="PSUM") as ps_m,
        ):
            # ---------------- loads ----------------
            # Direct DMA loads: Bacc's generate_event_semaphores splits
            # multi-semaphore waits on consumers, so no DVE staging needed.
            def staged_load(name, shape, dt, src_ap):
                sb = const.tile(shape, dt, name=f"{name}_sb")
                nc.sync.dma_start(sb[:], src_ap)
                return sb

            # x split per token-tile so each transpose only waits on its
            # own slice's DMA
            x_sb = const.tile([P, TT, C], F32)
            x_r = x_d[:].rearrange("(t p) c -> p t c", p=P)
            for tt in range(TT):
                nc.sync.dma_start(x_sb[:, tt, :], x_r[:, tt, :])
            wq_sb = staged_load(
                "wq", [P, 2, CC, P], MD,
                wq_d[:].rearrange("g (cc p) f -> p g cc f", p=P),
            )
            wk_sb = staged_load(
                "wk", [P, 2, CC, P], MD,
                wk_d[:].rearrange("g (cc p) f -> p g cc f", p=P),
            )
            wv_sb = staged_load(
                "wv", [P, CC, P], MD, wv_d[:].rearrange("(cc p) f -> p cc f", p=P)
            )
            wp_sb = staged_load("wp", [P, 2, C], PD, wp_d[:].rearrange("g p c -> p g c"))
            sel_sb = staged_load("sel", [P, P], PD, sel_d[:])
            idn_sb = staged_load("idn", [P, P], F32, idn_d[:])
            bq_sb = staged_load("bq", [P, 2], F32, bq_d[:].rearrange("g p -> p g"))
            bk_sb = staged_load("bk", [P, 2], F32, bk_d[:].rearrange("g p -> p g"))
            bv_sb = staged_load(
                "bv", [P, 1], F32, bv_d[:].rearrange("(p o) -> p o", o=1)
            )

            # reps>1: wrap the whole compute in a hardware loop so device
            # time dominates host/dispatch overhead for timing runs
            from contextlib import nullcontext

            loop_ctx = tc.For_i(0, reps, 1) if reps > 1 else nullcontext()
            with loop_ctx:
                _build_body(
                    nc, tc, const, work, ptp, ps_s, ps_m,
                    N, KT, QC, NQ, TT, MD, AD, PD,
                    x_sb, wq_sb, wk_sb, wv_sb, wp_sb, sel_sb, idn_sb,
                    bq_sb, bk_sb, bv_sb, out_d,
                )
    nc.finalize()
    return nc


def _build_body(
    nc, tc, const, work, ptp, ps_s, ps_m,
    N, KT, QC, NQ, TT, MD, AD, PD,
    x_sb, wq_sb, wk_sb, wv_sb, wp_sb, sel_sb, idn_sb,
    bq_sb, bk_sb, bv_sb, out_d,
):
    if True:
        if True:
            # ---------------- xT via PE transpose ----------------
            xt_sb = const.tile([P, CC, N], MD)
            for tt in range(TT):
                for cc in range(CC):
                    tp = ps_m.tile([P, P], F32, tag="misc", name="tp")
                    nc.tensor.transpose(
                        tp[:], x_sb[:, tt, cc * P : (cc + 1) * P], idn_sb[:]
                    )
                    nc.vector.tensor_copy(xt_sb[:, cc, tt * P : (tt + 1) * P], tp[:])

            # ---------------- qkv projections ----------------
            qt_sb = const.tile([P, 2, N], MD)
            kt_sb = const.tile([P, 2, N], MD)
            vt_sb = const.tile([P, N], F32)
            for g2 in range(2):
                for w_sb, b_sb, dst in ((wq_sb, bq_sb, qt_sb), (wk_sb, bk_sb, kt_sb)):
                    for nn in range(N // QC):
                        ps = ps_m.tile([P, QC], F32, tag="misc", name="ps")
                        for cc in range(CC):
                            nc.tensor.matmul(
                                ps[:],
                                w_sb[:, g2, cc, :],
                                xt_sb[:, cc, nn * QC : (nn + 1) * QC],
                                start=(cc == 0),
                                stop=(cc == CC - 1),
                            )
                        # copy + in-place add: TensorScalar's ISA struct only
                        # fits one sync wait, so it must not read PSUM (PE
                        # wait) and carry its DVE pipeline wait at once
                        dslice = dst[:, g2, nn * QC : (nn + 1) * QC]
                        nc.vector.tensor_copy(dslice, ps[:])
                        nc.vector.tensor_scalar_add(dslice, dslice, b_sb[:, g2 : g2 + 1])
            for nn in range(N // QC):
                ps = ps_m.tile([P, QC], F32, tag="misc", name="ps")
                for cc in range(CC):
                    nc.tensor.matmul(
                        ps[:],
                        wv_sb[:, cc, :],
                        xt_sb[:, cc, nn * QC : (nn + 1) * QC],
                        start=(cc == 0),
                        stop=(cc == CC - 1),
                    )
                vslice = vt_sb[:, nn * QC : (nn + 1) * QC]
                nc.vector.tensor_copy(vslice, ps[:])
                nc.vector.tensor_scalar_add(vslice, vslice, bv_sb[:, 0:1])

            # ---------------- v_aug (v natural layout + ones column) -------
            vaug = const.tile([P, KT, 8, 17], AD)
            # fp32r memset has no ISA encoding; broadcast-copy 1.0 from an
            # fp32 tile instead (copies round to fp32r)
            ones_sb = const.tile([P, 1], F32)
            nc.vector.memset(ones_sb[:], 1.0)
            zeros_sb = const.tile([P, 1], F32)
            nc.vector.memset(zeros_sb[:], 0.0)
            nc.vector.tensor_copy(
                vaug[:, :, :, 16], ones_sb[:, 0:1, None].to_broadcast((P, KT, 8))
            )
            for kt in range(KT):
                tp = ps_m.tile([P, P], F32, tag="misc", name="tp")
                nc.tensor.transpose(tp[:], vt_sb[:, kt * P : (kt + 1) * P], idn_sb[:])
                nc.vector.tensor_copy(
                    vaug[:, kt, :, 0:16], tp[:].rearrange("p (h d) -> p h d", d=16)
                )

            # ---------------- attention ----------------
            for nn in range(NQ):
                ot_n = work.tile([P, 2, QC], PD, tag="otn")
                for g2 in range(2):
                    # one accumulator bank per head: independent psum
                    # accumulation chains must not share a zero region
                    at = [
                        ps_m.tile([P, QC], F32, tag="misc", name=f"at{_lj}")
                        for _lj in range(4)
                    ]
                    for kt in range(KT):
                        for pr in range(2):
                            sc = ps_s.tile([P, 2 * QC], F32, tag="scores", name="sc")
                            for j2 in range(2):
                                lj = 2 * pr + j2
                                rg = 32 * lj
                                nc.tensor.matmul(
                                    sc[:, j2 * QC : (j2 + 1) * QC],
                                    kt_sb[rg : rg + D, g2, kt * P : (kt + 1) * P],
                                    qt_sb[rg : rg + D, g2, nn * QC : (nn + 1) * QC],
                                    start=True,
                                    stop=True,
                                    tile_position=(rg, 0),
                                )
                            pt = ptp.tile([P, 2 * QC], AD, tag="pt", name="pt")
                            nc.scalar.activation(pt[:], sc[:], EXPF)
                            for j2 in range(2):
                                lj = 2 * pr + j2
                                nc.tensor.matmul(
                                    at[lj][32 * lj : 32 * lj + 17, :],
                                    vaug[:, kt, 4 * g2 + lj, :],
                                    pt[:, j2 * QC : (j2 + 1) * QC],
                                    start=(kt == 0),
                                    stop=(kt == KT - 1),
                                    tile_position=(0, 32 * lj),
                                )
                    # normalize: broadcast sums over each col-group, divide
                    ot_raw = work.tile([P, QC], PD, tag="otraw")
                    # (fp32r memset unsupported: broadcast-copy zero instead)
                    nc.vector.tensor_copy(
                        ot_raw[:], zeros_sb[:, 0:1].to_broadcast((P, QC))
                    )
                    for lj in range(4):
                        nc.vector.tensor_copy(
                            ot_raw[32 * lj : 32 * lj + 17, :],
                            at[lj][32 * lj : 32 * lj + 17, :],
                        )
                    bc = ps_m.tile([P, QC], F32, tag="misc", name="bc")
                    nc.tensor.matmul(
                        bc[:], sel_sb[:], ot_raw[:], start=True, stop=True
                    )
                    rec = work.tile([P, QC], F32, tag="rec")
                    nc.vector.reciprocal(rec[:], bc[:])
                    nc.vector.tensor_mul(ot_n[:, g2, :], ot_raw[:], rec[:])
                # output projection for this q-chunk
                for ss in range(QC // P):
                    pp = ps_m.tile([P, C], F32, tag="misc", name="pp")
                    for g2 in range(2):
                        nc.tensor.matmul(
                            pp[:],
                            ot_n[:, g2, ss * P : (ss + 1) * P],
                            wp_sb[:, g2, :],
                            start=(g2 == 0),
                            stop=(g2 == 1),
                        )
                    ob = work.tile([P, C], F32, tag="ob")
                    nc.vector.tensor_copy(ob[:], pp[:])
                    tt_idx = nn * (QC // P) + ss
                    nc.sync.dma_start(
                        out_d[:].rearrange("(t p) c -> p t c", p=P)[:, tt_idx, :],
                        ob[:],
                    )


def _get_nc(n_tokens=N_FULL, reps=1):
    key = (n_tokens, MM_DT, AV_DT, PROJ_DT, reps)
    if key not in _NC_CACHE:
        _NC_CACHE[key] = build(n_tokens, MM_DT, AV_DT, PROJ_DT, reps=reps)
    return _NC_CACHE[key]


def make_core_inputs(core, x, w_qkv, b_qkv, w_proj, n_tokens=N_FULL):
    """Host-side sharding: slice/spread weights for one core."""
    b, g = core // 2, core % 2
    wq_s = np.zeros((2, C, P), np.float32)
    wk_s = np.zeros((2, C, P), np.float32)
    bq_s = np.zeros((2, P), np.float32)
    bk_s = np.zeros((2, P), np.float32)
    wv_s = np.zeros((C, P), np.float32)
    bv_s = np.zeros((P,), np.float32)
    wp_s = np.zeros((2, P, C), np.float32)
    for g2 in range(2):
        for j in range(4):
            h = 8 * g + 4 * g2 + j
            sp = slice(32 * j, 32 * j + D)
            wq_s[g2, :, sp] = w_qkv[:, 0 * C + h * D : 0 * C + (h + 1) * D]
            wk_s[g2, :, sp] = w_qkv[:, 1 * C + h * D : 1 * C + (h + 1) * D]
            bq_s[g2, sp] = b_qkv[0 * C + h * D : 0 * C + (h + 1) * D]
            bk_s[g2, sp] = b_qkv[1 * C + h * D : 1 * C + (h + 1) * D]
            wp_s[g2, sp, :] = w_proj[h * D : (h + 1) * D, :]
    for lh in range(8):
        h = 8 * g + lh
        wv_s[:, 16 * lh : 16 * lh + 16] = w_qkv[:, 2 * C + h * D : 2 * C + (h + 1) * D]
        bv_s[16 * lh : 16 * lh + 16] = b_qkv[2 * C + h * D : 2 * C + (h + 1) * D]
    sel = np.zeros((P, P), np.float32)
    for j in range(4):
        sel[32 * j + 16, 32 * j : 32 * j + 32] = 1.0
    idn = np.eye(P, dtype=np.float32)

    def cast(a, stage_dt):
        if stage_dt == "bf16":
            import ml_dtypes
            return a.astype(ml_dtypes.bfloat16)
        return a.astype(np.float32)

    return {
        "x": np.ascontiguousarray(x[b, :n_tokens], dtype=np.float32),
        "wq": cast(wq_s, MM_DT), "wk": cast(wk_s, MM_DT), "wv": cast(wv_s, MM_DT),
        "bq": bq_s, "bk": bk_s, "bv": bv_s,
        "wp": cast(wp_s, PROJ_DT), "sel": cast(sel, PROJ_DT), "idn": idn,
    }


def kernel(x, w_qkv, b_qkv, w_proj, b_proj):
    global LAST_RESULT
    from concourse.bass_utils import run_bass_kernel_spmd

    x = np.asarray(x, dtype=np.float32)
    w_qkv = np.asarray(w_qkv, dtype=np.float32)
    b_qkv = np.asarray(b_qkv, dtype=np.float32)
    w_proj = np.asarray(w_proj, dtype=np.float32)
    b_proj = np.asarray(b_proj, dtype=np.float32)

    nc = _get_nc(reps=TIMING_REPS)
    in_maps = [
        make_core_inputs(core, x, w_qkv, b_qkv, w_proj) for core in range(NCORES)
    ]
    res = run_bass_kernel_spmd(nc, in_maps, list(range(NCORES)))
    LAST_RESULT = res
    out = np.zeros((B, N_FULL, C), np.float32)
    for core in range(NCORES):
        out[core // 2] += res.results[core]["out"]
    out += b_proj[None, None, :]
    return out
